# revision 15
# baseline (speedup 1.0000x reference)
"""Trainium2 Bass kernel for AdjacencyMatchingLoss (8-core SPMD).

Math: adj_score[b,e] = P[b,i_e,:] @ A @ P[b,j_e,:]  with A = (d_hw==1).
Let W[i,j] = sum_e w_e * 1[i_e=i] * 1[j_e=j]   (weighted pair histogram)
and Gm = sum_b P_b A P_b^T scaled by -1/8 (sign + batch mean folded into
the A mask). Then the per-core partial numerator is <W, Gm>.

Structure (v2 — "Gm-first + host one-hot streaming"):
- Host ships PT (P transposed to [q, b*l], bf16). With A in natural [q,r]
  layout, Z_b = matmul(lhsT=Asc, rhs=PT_b) = (P_b A)^T and
  G_b = matmul(lhsT=Z_b, rhs=PT_b) = P_b A P_b^T — no on-device
  transposes. Gm accumulates over b in one PSUM group while the one-hot
  stream is still in flight.
- Host ships the edge one-hots directly as fp8e4m3 ([e-chunk layout,
  128-wide rows]): OhIW carries w * onehot(i), OhJ carries onehot(j)
  (exact 0/1 in fp8). One fused tensor, piece-interleaved
  [IW_p | J_p | ...] so each DMA piece delivers both matmul operands for
  a run of chunks; the PE consumes pieces as they land using DoubleRow
  fp8 matmuls (K=256: two 128-edge chunks per instruction, 0.5
  cycles/row).
- Tail: <W, Gm> = one DVE multiply (W in PSUM x Gm in SBUF) + one
  reduce, then a [128,2] partials DMA ([numerator partial, sum(w)]);
  the host sums partials over partitions/cores and divides (that
  reduction is part of unsharding the scalar output).

The w values ride inside OhIW in fp8 (~2% per-edge rounding, random
sign, averages out over 50k edges: final rel err ~1e-4). P in bf16.

This replaced a DVE-built one-hot design (21us of DVE TensorTensor at
1x — broadcast operands disqualify the 2x/4x DVE modes). CoreSim for
this version predicts ~7us vs 29.5us for the old one.
"""

import os
import sys

import numpy as np

for _p in ("/opt/trn_rl_repo",):
    if os.path.isdir(_p) and _p not in sys.path:
        sys.path.insert(0, _p)

B, NL, NQ, E = 8, 128, 128, 50000
NCORES = 8
ESH = E // NCORES            # 6250 edges per core
CHUNKS = (ESH + 127) // 128  # 49
EPAD = CHUNKS * 128          # 6272

# one-hot stream pieces, in chunk-PAIR units (DoubleRow consumes pairs);
# last piece has the odd single chunk appended
PIECE_CHUNKS = [(0, 14), (14, 26), (26, 38), (38, 49)]

PT_FP8 = False
# pm_in packs PT + meta into ONE byte tensor [128, PM_W] (single DMA):
#   PT_FP8: [0:1024)B PT fp8 | [1024:1122)B w bf16 | [1122:1250)B d int8
#   else:   [0:2048)B PT bf16 | [2048:2146)B w bf16 | [2146:2274)B d int8
PT_B = 1024 if PT_FP8 else 2048
PM_W = PT_B + 98 + 128
BUFS = 1

_BUILT = None


def _emit_body(nc, sp, pp, tensors):
    import concourse.mybir as mybir

    f32 = mybir.dt.float32
    bf16 = mybir.dt.bfloat16
    i16 = mybir.dt.int16
    i8 = mybir.dt.int8
    fp8 = mybir.dt.float8e4
    EQ = mybir.AluOpType.is_equal
    MUL = mybir.AluOpType.mult
    ADD = mybir.AluOpType.add
    DR = mybir.MatmulPerfMode.DoubleRow
    pm_d, oh_d, o_d = tensors

    pm = sp.tile([128, PM_W], i8)
    oh = sp.tile([128, 2 * EPAD], fp8)
    Asc = sp.tile([128, NQ], bf16)
    Zsb = sp.tile([128, B * NL], bf16)
    GmS = sp.tile([128, NL], bf16)
    scr = sp.tile([128, NL], f32)
    prt = sp.tile([128, 2], f32)

    Zps = pp.tile([128, B * NL], f32)
    Gps = pp.tile([128, NL], f32)
    Wps = pp.tile([128, NL], f32)

    # ---- DMAs (emitted up front so transfers stream back-to-back) ----
    nc.sync.dma_start(out=pm[:], in_=pm_d.ap())
    piece_off = []
    off = 0
    for c0, c1 in PIECE_CHUNKS:
        sz = (c1 - c0) * 128
        piece_off.append(off)
        nc.sync.dma_start(
            out=oh[:, off : off + 2 * sz], in_=oh_d.ap()[:, off : off + 2 * sz]
        )
        off += 2 * sz

    def iw_ap(c):
        for (c0, c1), po in zip(PIECE_CHUNKS, piece_off):
            if c0 <= c < c1:
                return po + (c - c0) * 128
        raise AssertionError(c)

    def j_ap(c):
        for (c0, c1), po in zip(PIECE_CHUNKS, piece_off):
            if c0 <= c < c1:
                return po + (c1 - c0) * 128 + (c - c0) * 128
        raise AssertionError(c)

    # views into pm (byte offsets)
    PT = pm[:, 0:PT_B].bitcast(fp8 if PT_FP8 else bf16)  # [128, 1024]
    wT = pm[:, PT_B : PT_B + 98].bitcast(bf16)           # [128, 49]
    dsb = pm[:, PT_B + 98 : PT_B + 226]                  # [128, 128] int8

    # ---- small prep ----
    # Asc = -(1/8) * (d_hw == 1): folds sign + batch-mean into the mask
    nc.gpsimd.tensor_scalar(
        out=Asc[:], in0=dsb, scalar1=1, scalar2=-0.125, op0=EQ, op1=MUL
    )
    nc.vector.tensor_reduce(
        out=prt[:, 1:2], in_=wT, axis=mybir.AxisListType.X, op=ADD
    )

    # ---- Gm = sum_b P_b Asc P_b^T via PT-only matmuls ----
    for b in range(B):
        sl = slice(b * 128, (b + 1) * 128)
        nc.tensor.matmul(
            Zps[:, sl], lhsT=Asc[:], rhs=PT[:, sl], start=True, stop=True
        )
    nc.vector.tensor_copy(out=Zsb[:, 0:512], in_=Zps[:, 0:512])
    nc.scalar.copy(out=Zsb[:, 512:1024], in_=Zps[:, 512:1024])
    for b in range(B):
        sl = slice(b * 128, (b + 1) * 128)
        nc.tensor.matmul(
            Gps[:], lhsT=Zsb[:, sl], rhs=PT[:, sl],
            start=(b == 0), stop=(b == B - 1),
        )
    nc.scalar.copy(out=GmS[:], in_=Gps[:])

    # ---- W accumulation from the one-hot stream ----
    c = 0
    while c < CHUNKS:
        if c + 1 < CHUNKS and iw_ap(c + 1) == iw_ap(c) + 128:
            two = lambda a: oh[:, a : a + 256].rearrange(
                "p (two m) -> p two m", two=2
            )
            nc.tensor.matmul(
                Wps[:], lhsT=two(iw_ap(c)), rhs=two(j_ap(c)),
                start=(c == 0), stop=(c + 2 >= CHUNKS),
                perf_mode=DR,
            )
            c += 2
        else:
            nc.tensor.matmul(
                Wps[:], lhsT=oh[:, iw_ap(c) : iw_ap(c) + 128],
                rhs=oh[:, j_ap(c) : j_ap(c) + 128],
                start=(c == 0), stop=(c + 1 >= CHUNKS),
            )
            c += 1

    # ---- tail: partial = sum_j W[p,j] * Gm[p,j] ----
    nc.vector.tensor_tensor(out=scr[:], in0=Wps[:], in1=GmS[:], op=MUL)
    nc.vector.tensor_reduce(
        out=prt[:, 0:1], in_=scr[:], axis=mybir.AxisListType.X, op=ADD
    )
    nc.sync.dma_start(out=o_d.ap(), in_=prt[:])


def _build(reps=1):
    import concourse.bacc as bacc
    import concourse.mybir as mybir
    import concourse.tile as tile

    f32 = mybir.dt.float32
    bf16 = mybir.dt.bfloat16
    i16 = mybir.dt.int16
    fp8 = mybir.dt.float8e4

    nc = bacc.Bacc("TRN2", target_bir_lowering=False, debug=False, num_devices=NCORES)

    pm_d = nc.dram_tensor("pm_in", [128, PM_W], mybir.dt.int8, kind="ExternalInput")
    oh_d = nc.dram_tensor("oh_in", [128, 2 * EPAD], fp8, kind="ExternalInput")
    o_d = nc.dram_tensor("out", [128, 2], f32, kind="ExternalOutput")

    with tile.TileContext(nc) as tc:
        with (
            tc.tile_pool(name="sbuf", bufs=BUFS) as sp,
            tc.tile_pool(name="psum", bufs=BUFS, space="PSUM") as pp,
        ):
            for _ in range(reps):
                _emit_body(nc, sp, pp, (pm_d, oh_d, o_d))

    nc.compile()
    return nc


def _get_built():
    global _BUILT
    if _BUILT is None:
        _BUILT = _build()
    return _BUILT


def _shard_inputs(P, d_hw, circuit_edge_pairs, circuit_edge_weights):
    import ml_dtypes

    bf16 = ml_dtypes.bfloat16
    fp8 = ml_dtypes.float8_e4m3

    P = np.asarray(P, dtype=np.float32)
    d = np.asarray(d_hw, dtype=np.int32)
    pairs = np.asarray(circuit_edge_pairs).astype(np.int64, copy=False)
    w = np.asarray(circuit_edge_weights, dtype=np.float32)

    # PT[q, b*128 + l] = P[b, l, q]  (replicated to all cores)
    PT = np.ascontiguousarray(P.transpose(2, 0, 1).reshape(128, B * NL))

    pairs_pad = np.zeros((NCORES, EPAD, 2), dtype=np.int64)
    w_pad = np.zeros((NCORES, EPAD), dtype=np.float32)
    pairs_pad[:, :ESH] = pairs.reshape(NCORES, ESH, 2)
    w_pad[:, :ESH] = w.reshape(NCORES, ESH)

    # edge k (per core) -> chunk cc = k//128, partition p = k%128
    k = np.arange(EPAD)
    cc = k // 128
    p = k % 128
    i_idx = pairs_pad[:, :, 0].astype(np.int64)
    j_idx = pairs_pad[:, :, 1].astype(np.int64)
    core = np.repeat(np.arange(NCORES), EPAD).reshape(NCORES, EPAD)
    pp_b = np.broadcast_to(p, (NCORES, EPAD))

    ohiw_full = np.zeros((NCORES, 128, EPAD), dtype=fp8)
    ohj_full = np.zeros((NCORES, 128, EPAD), dtype=fp8)
    ohiw_full[core, pp_b, cc * 128 + i_idx] = w_pad.astype(fp8)
    ohj_full[core, pp_b, cc * 128 + j_idx] = fp8(1.0)

    # fuse into piece-interleaved layout [IW_piece | J_piece | ...]
    oh = np.zeros((NCORES, 128, 2 * EPAD), dtype=fp8)
    off = 0
    for c0, c1 in PIECE_CHUNKS:
        sz = (c1 - c0) * 128
        oh[:, :, off : off + sz] = ohiw_full[:, :, c0 * 128 : c1 * 128]
        oh[:, :, off + sz : off + 2 * sz] = ohj_full[:, :, c0 * 128 : c1 * 128]
        off += 2 * sz

    # pm: PT | w | d_hw(int8), byte layout
    w_b = w_pad.reshape(NCORES, CHUNKS, 128).transpose(0, 2, 1).astype(bf16)
    d8 = d.astype(np.int8)  # values 0..3 fit

    pm = np.zeros((NCORES, 128, PM_W), dtype=np.int8)
    pm[:, :, 0:PT_B] = PT.astype(fp8 if PT_FP8 else bf16).view(np.int8)[None]
    pm[:, :, PT_B : PT_B + 98] = np.ascontiguousarray(w_b).view(np.int8)
    pm[:, :, PT_B + 98 : PT_B + 226] = d8.view(np.int8)[None]

    return [
        {
            "pm_in": np.ascontiguousarray(pm[i]),
            "oh_in": np.ascontiguousarray(oh[i]),
        }
        for i in range(NCORES)
    ]


def _combine(results):
    parts = np.stack([np.asarray(results[i]["out"]) for i in range(NCORES)])
    numer = float(parts[:, :, 0].astype(np.float64).sum())
    wsum = float(parts[:, :, 1].astype(np.float64).sum())
    return np.float32(numer / max(wsum, 1e-8))


def make_runner(nc, n_cores=NCORES):
    """jit-once mirror of bass2jax.run_bass_via_pjrt's multi-core branch so
    repeated kernel() calls reuse the compiled NEFF."""
    import jax
    import concourse.mybir as mybir
    from concourse.bass2jax import (
        Mesh,
        PartitionSpec,
        _bass_exec_p,
        install_neuronx_cc_hook,
        partition_id_tensor,
        shard_map,
    )

    install_neuronx_cc_hook()
    partition_name = nc.partition_id_tensor.name if nc.partition_id_tensor else None

    in_names, out_names, out_avals, zero_outs = [], [], [], []
    for alloc in nc.m.functions[0].allocations:
        if not isinstance(alloc, mybir.MemoryLocationSet):
            continue
        name = alloc.memorylocations[0].name
        if alloc.kind == "ExternalInput":
            if name != partition_name:
                in_names.append(name)
        elif alloc.kind == "ExternalOutput":
            shape = tuple(alloc.tensor_shape)
            dtype = mybir.dt.np(alloc.dtype)
            out_names.append(name)
            out_avals.append(jax.core.ShapedArray(shape, dtype))
            zero_outs.append(np.zeros(shape, dtype))
    n_params = len(in_names)
    n_outs = len(out_avals)
    all_names = in_names + out_names
    if partition_name is not None:
        all_names = all_names + [partition_name]
    donate = tuple(range(n_params, n_params + n_outs))

    def _body(*args):
        operands = list(args)
        if partition_name is not None:
            operands.append(partition_id_tensor())
        outs = _bass_exec_p.bind(
            *operands,
            out_avals=tuple(out_avals),
            in_names=tuple(all_names),
            out_names=tuple(out_names),
            lowering_input_output_aliases=(),
            sim_require_finite=True,
            sim_require_nnan=True,
            nc=nc,
        )
        return tuple(outs)

    devices = jax.devices()[:n_cores]
    mesh = Mesh(np.asarray(devices), ("core",))
    sharded = jax.jit(
        shard_map(
            _body,
            mesh=mesh,
            in_specs=(PartitionSpec("core"),) * (n_params + n_outs),
            out_specs=(PartitionSpec("core"),) * n_outs,
            check_rep=False,
        ),
        donate_argnums=donate,
        keep_unused=True,
    )

    def prep(in_maps):
        concat_in = [
            np.concatenate([np.asarray(m[name]) for m in in_maps], axis=0)
            for name in in_names
        ]
        return [jax.device_put(a) for a in concat_in]

    def run_dev(dev_in):
        concat_zeros = [
            np.zeros((n_cores * z.shape[0], *z.shape[1:]), z.dtype)
            for z in zero_outs
        ]
        out_arrs = sharded(*dev_in, *concat_zeros)
        out_arrs = [np.asarray(a) for a in out_arrs]
        return [
            {
                name: out_arrs[i].reshape(n_cores, *out_avals[i].shape)[c]
                for i, name in enumerate(out_names)
            }
            for c in range(n_cores)
        ]

    def run(in_maps):
        return run_dev(prep(in_maps))

    run.prep = prep
    run.run_dev = run_dev
    return run


_RUNNER = None


def kernel(P, d_hw, circuit_edge_pairs, circuit_edge_weights, _want_results=False):
    global _RUNNER
    in_maps = _shard_inputs(P, d_hw, circuit_edge_pairs, circuit_edge_weights)
    try:
        if _RUNNER is None:
            _RUNNER = make_runner(_get_built())
        results = _RUNNER(in_maps)
        res = None
    except Exception:
        if _want_results:
            raise
        # fallback: the stock SPMD runner (recompiles per call, but robust)
        from concourse.bass_utils import run_bass_kernel_spmd

        res = run_bass_kernel_spmd(
            _get_built(), in_maps, core_ids=list(range(NCORES))
        )
        results = res.results
    out = _combine(results)
    if _want_results:
        return out, res
    return out


# revision 19
# speedup vs baseline: 1.3920x; 1.3920x over previous
"""Trainium2 Bass kernel for AdjacencyMatchingLoss (8-core SPMD).

Math: adj_score[b,e] = P[b,i_e,:] @ A @ P[b,j_e,:]  with A = (d_hw==1).
Let W[i,j] = sum_e w_e * 1[i_e=i] * 1[j_e=j]   (weighted pair histogram)
and Gm = sum_b P_b A P_b^T scaled by -1/8 (sign + batch mean folded into
the A mask). Then the per-core partial numerator is <W, Gm>.

Structure (v2 — "Gm-first + host one-hot streaming"):
- Host ships PT (P transposed to [q, b*l], bf16). With A in natural [q,r]
  layout, Z_b = matmul(lhsT=Asc, rhs=PT_b) = (P_b A)^T and
  G_b = matmul(lhsT=Z_b, rhs=PT_b) = P_b A P_b^T — no on-device
  transposes. Gm accumulates over b in one PSUM group while the one-hot
  stream is still in flight.
- Host ships the edge one-hots directly as fp8e4m3 ([e-chunk layout,
  128-wide rows]): OhIW carries w * onehot(i), OhJ carries onehot(j)
  (exact 0/1 in fp8). One fused tensor, piece-interleaved
  [IW_p | J_p | ...] so each DMA piece delivers both matmul operands for
  a run of chunks; the PE consumes pieces as they land using DoubleRow
  fp8 matmuls (K=256: two 128-edge chunks per instruction, 0.5
  cycles/row).
- Tail: <W, Gm> = one DVE multiply (W in PSUM x Gm in SBUF) + one
  reduce, then a [128,2] partials DMA ([numerator partial, sum(w)]);
  the host sums partials over partitions/cores and divides (that
  reduction is part of unsharding the scalar output).

The w values ride inside OhIW in fp8 (~2% per-edge rounding, random
sign, averages out over 50k edges: final rel err ~1e-4). P in bf16.

This replaced a DVE-built one-hot design (21us of DVE TensorTensor at
1x — broadcast operands disqualify the 2x/4x DVE modes). CoreSim for
this version predicts ~7us vs 29.5us for the old one.
"""

import os
import sys

import numpy as np

for _p in ("/opt/trn_rl_repo",):
    if os.path.isdir(_p) and _p not in sys.path:
        sys.path.insert(0, _p)

B, NL, NQ, E = 8, 128, 128, 50000
NCORES = 8
ESH = E // NCORES            # 6250 edges per core
CHUNKS = (ESH + 127) // 128  # 49
EPAD = CHUNKS * 128          # 6272

# one-hot stream pieces, in chunk-PAIR units (DoubleRow consumes pairs);
# last piece has the odd single chunk appended
PIECE_CHUNKS = [(0, 14), (14, 26), (26, 38), (38, 49)]

PT_FP8 = False
# pm_in packs PT + meta into ONE byte tensor [128, PM_W] (single DMA):
#   PT_FP8: [0:1024)B PT fp8 | [1024:1122)B w bf16 | [1122:1250)B d int8
#   else:   [0:2048)B PT bf16 | [2048:2146)B w bf16 | [2146:2274)B d int8
PT_B = 1024 if PT_FP8 else 2048
PM_W = PT_B + 98 + 128
# SBUF double-buffered so rep n+1's DMAs overlap rep n's compute; PSUM
# stays single-buffered (bufs=2 PSUM was HW-incorrect at high rep counts
# despite passing CoreSim — see transcript; 8/8 banks + cross-rep
# accumulation-group interleaving is the suspect).
SBUF_BUFS = 2
PSUM_BUFS = 1
DIAG = None  # None | 'dma' (DMAs only) | 'nogm' (skip Gm chain)

_BUILT = None


def _emit_body(nc, sp, pp, tensors):
    import concourse.mybir as mybir

    f32 = mybir.dt.float32
    bf16 = mybir.dt.bfloat16
    i16 = mybir.dt.int16
    i8 = mybir.dt.int8
    fp8 = mybir.dt.float8e4
    EQ = mybir.AluOpType.is_equal
    MUL = mybir.AluOpType.mult
    ADD = mybir.AluOpType.add
    DR = mybir.MatmulPerfMode.DoubleRow
    pm_d, oh_d, o_d = tensors

    pm = sp.tile([128, PM_W], i8)
    oh = sp.tile([128, 2 * EPAD], fp8)
    Asc = sp.tile([128, NQ], bf16)
    Zsb = sp.tile([128, B * NL], bf16)
    GmS = sp.tile([128, NL], bf16)
    scr = sp.tile([128, NL], f32)
    prt = sp.tile([128, 2], f32)

    Zps = pp.tile([128, B * NL], f32)
    Gps = pp.tile([128, NL], f32)
    Wps = pp.tile([128, NL], f32)

    # ---- DMAs (emitted up front so transfers stream back-to-back) ----
    nc.sync.dma_start(out=pm[:], in_=pm_d.ap())
    piece_off = []
    off = 0
    for c0, c1 in PIECE_CHUNKS:
        sz = (c1 - c0) * 128
        piece_off.append(off)
        nc.sync.dma_start(
            out=oh[:, off : off + 2 * sz], in_=oh_d.ap()[:, off : off + 2 * sz]
        )
        off += 2 * sz

    def iw_ap(c):
        for (c0, c1), po in zip(PIECE_CHUNKS, piece_off):
            if c0 <= c < c1:
                return po + (c - c0) * 128
        raise AssertionError(c)

    def j_ap(c):
        for (c0, c1), po in zip(PIECE_CHUNKS, piece_off):
            if c0 <= c < c1:
                return po + (c1 - c0) * 128 + (c - c0) * 128
        raise AssertionError(c)

    # views into pm (byte offsets)
    PT = pm[:, 0:PT_B].bitcast(fp8 if PT_FP8 else bf16)  # [128, 1024]
    wT = pm[:, PT_B : PT_B + 98].bitcast(bf16)           # [128, 49]
    dsb = pm[:, PT_B + 98 : PT_B + 226]                  # [128, 128] int8

    if DIAG == "dma":
        # DMA-floor probe: touch the inputs with one cheap op, then out
        nc.vector.tensor_reduce(
            out=prt[:, 1:2], in_=wT, axis=mybir.AxisListType.X, op=ADD
        )
        nc.vector.tensor_reduce(
            out=prt[:, 0:1], in_=oh[:, 0:128].bitcast(i8),
            axis=mybir.AxisListType.X, op=ADD,
        )
        nc.sync.dma_start(out=o_d.ap(), in_=prt[:])
        return

    # ---- small prep ----
    # Asc = -(1/8) * (d_hw == 1): folds sign + batch-mean into the mask
    nc.gpsimd.tensor_scalar(
        out=Asc[:], in0=dsb, scalar1=1, scalar2=-0.125, op0=EQ, op1=MUL
    )
    nc.vector.tensor_reduce(
        out=prt[:, 1:2], in_=wT, axis=mybir.AxisListType.X, op=ADD
    )

    # ---- Gm = sum_b P_b Asc P_b^T via PT-only matmuls ----
    for b in range(B):
        sl = slice(b * 128, (b + 1) * 128)
        nc.tensor.matmul(
            Zps[:, sl], lhsT=Asc[:], rhs=PT[:, sl], start=True, stop=True
        )
    nc.vector.tensor_copy(out=Zsb[:, 0:512], in_=Zps[:, 0:512])
    nc.scalar.copy(out=Zsb[:, 512:1024], in_=Zps[:, 512:1024])
    for b in range(B):
        sl = slice(b * 128, (b + 1) * 128)
        nc.tensor.matmul(
            Gps[:], lhsT=Zsb[:, sl], rhs=PT[:, sl],
            start=(b == 0), stop=(b == B - 1),
        )
    nc.scalar.copy(out=GmS[:], in_=Gps[:])

    # ---- W accumulation from the one-hot stream ----
    c = 0
    while c < CHUNKS:
        if c + 1 < CHUNKS and iw_ap(c + 1) == iw_ap(c) + 128:
            two = lambda a: oh[:, a : a + 256].rearrange(
                "p (two m) -> p two m", two=2
            )
            nc.tensor.matmul(
                Wps[:], lhsT=two(iw_ap(c)), rhs=two(j_ap(c)),
                start=(c == 0), stop=(c + 2 >= CHUNKS),
                perf_mode=DR,
            )
            c += 2
        else:
            nc.tensor.matmul(
                Wps[:], lhsT=oh[:, iw_ap(c) : iw_ap(c) + 128],
                rhs=oh[:, j_ap(c) : j_ap(c) + 128],
                start=(c == 0), stop=(c + 1 >= CHUNKS),
            )
            c += 1

    # ---- tail: partial = sum_j W[p,j] * Gm[p,j] ----
    nc.vector.tensor_tensor(out=scr[:], in0=Wps[:], in1=GmS[:], op=MUL)
    nc.vector.tensor_reduce(
        out=prt[:, 0:1], in_=scr[:], axis=mybir.AxisListType.X, op=ADD
    )
    nc.sync.dma_start(out=o_d.ap(), in_=prt[:])


def _build(reps=1):
    import concourse.bacc as bacc
    import concourse.mybir as mybir
    import concourse.tile as tile

    f32 = mybir.dt.float32
    bf16 = mybir.dt.bfloat16
    i16 = mybir.dt.int16
    fp8 = mybir.dt.float8e4

    nc = bacc.Bacc("TRN2", target_bir_lowering=False, debug=False, num_devices=NCORES)

    pm_d = nc.dram_tensor("pm_in", [128, PM_W], mybir.dt.int8, kind="ExternalInput")
    oh_d = nc.dram_tensor("oh_in", [128, 2 * EPAD], fp8, kind="ExternalInput")
    o_d = nc.dram_tensor("out", [128, 2], f32, kind="ExternalOutput")

    with tile.TileContext(nc) as tc:
        with (
            tc.tile_pool(name="sbuf", bufs=SBUF_BUFS) as sp,
            tc.tile_pool(name="psum", bufs=PSUM_BUFS, space="PSUM") as pp,
        ):
            for _ in range(reps):
                _emit_body(nc, sp, pp, (pm_d, oh_d, o_d))

    nc.compile()
    return nc


def _get_built():
    global _BUILT
    if _BUILT is None:
        _BUILT = _build()
    return _BUILT


def _shard_inputs(P, d_hw, circuit_edge_pairs, circuit_edge_weights):
    import ml_dtypes

    bf16 = ml_dtypes.bfloat16
    fp8 = ml_dtypes.float8_e4m3

    P = np.asarray(P, dtype=np.float32)
    d = np.asarray(d_hw, dtype=np.int32)
    pairs = np.asarray(circuit_edge_pairs).astype(np.int64, copy=False)
    w = np.asarray(circuit_edge_weights, dtype=np.float32)

    # PT[q, b*128 + l] = P[b, l, q]  (replicated to all cores)
    PT = np.ascontiguousarray(P.transpose(2, 0, 1).reshape(128, B * NL))

    pairs_pad = np.zeros((NCORES, EPAD, 2), dtype=np.int64)
    w_pad = np.zeros((NCORES, EPAD), dtype=np.float32)
    pairs_pad[:, :ESH] = pairs.reshape(NCORES, ESH, 2)
    w_pad[:, :ESH] = w.reshape(NCORES, ESH)

    # edge k (per core) -> chunk cc = k//128, partition p = k%128
    k = np.arange(EPAD)
    cc = k // 128
    p = k % 128
    i_idx = pairs_pad[:, :, 0].astype(np.int64)
    j_idx = pairs_pad[:, :, 1].astype(np.int64)
    core = np.repeat(np.arange(NCORES), EPAD).reshape(NCORES, EPAD)
    pp_b = np.broadcast_to(p, (NCORES, EPAD))

    ohiw_full = np.zeros((NCORES, 128, EPAD), dtype=fp8)
    ohj_full = np.zeros((NCORES, 128, EPAD), dtype=fp8)
    ohiw_full[core, pp_b, cc * 128 + i_idx] = w_pad.astype(fp8)
    ohj_full[core, pp_b, cc * 128 + j_idx] = fp8(1.0)

    # fuse into piece-interleaved layout [IW_piece | J_piece | ...]
    oh = np.zeros((NCORES, 128, 2 * EPAD), dtype=fp8)
    off = 0
    for c0, c1 in PIECE_CHUNKS:
        sz = (c1 - c0) * 128
        oh[:, :, off : off + sz] = ohiw_full[:, :, c0 * 128 : c1 * 128]
        oh[:, :, off + sz : off + 2 * sz] = ohj_full[:, :, c0 * 128 : c1 * 128]
        off += 2 * sz

    # pm: PT | w | d_hw(int8), byte layout
    w_b = w_pad.reshape(NCORES, CHUNKS, 128).transpose(0, 2, 1).astype(bf16)
    d8 = d.astype(np.int8)  # values 0..3 fit

    pm = np.zeros((NCORES, 128, PM_W), dtype=np.int8)
    pm[:, :, 0:PT_B] = PT.astype(fp8 if PT_FP8 else bf16).view(np.int8)[None]
    pm[:, :, PT_B : PT_B + 98] = np.ascontiguousarray(w_b).view(np.int8)
    pm[:, :, PT_B + 98 : PT_B + 226] = d8.view(np.int8)[None]

    return [
        {
            "pm_in": np.ascontiguousarray(pm[i]),
            "oh_in": np.ascontiguousarray(oh[i]),
        }
        for i in range(NCORES)
    ]


def _combine(results):
    parts = np.stack([np.asarray(results[i]["out"]) for i in range(NCORES)])
    numer = float(parts[:, :, 0].astype(np.float64).sum())
    wsum = float(parts[:, :, 1].astype(np.float64).sum())
    return np.float32(numer / max(wsum, 1e-8))


def make_runner(nc, n_cores=NCORES):
    """jit-once mirror of bass2jax.run_bass_via_pjrt's multi-core branch so
    repeated kernel() calls reuse the compiled NEFF."""
    import jax
    import concourse.mybir as mybir
    from concourse.bass2jax import (
        Mesh,
        PartitionSpec,
        _bass_exec_p,
        install_neuronx_cc_hook,
        partition_id_tensor,
        shard_map,
    )

    install_neuronx_cc_hook()
    partition_name = nc.partition_id_tensor.name if nc.partition_id_tensor else None

    in_names, out_names, out_avals, zero_outs = [], [], [], []
    for alloc in nc.m.functions[0].allocations:
        if not isinstance(alloc, mybir.MemoryLocationSet):
            continue
        name = alloc.memorylocations[0].name
        if alloc.kind == "ExternalInput":
            if name != partition_name:
                in_names.append(name)
        elif alloc.kind == "ExternalOutput":
            shape = tuple(alloc.tensor_shape)
            dtype = mybir.dt.np(alloc.dtype)
            out_names.append(name)
            out_avals.append(jax.core.ShapedArray(shape, dtype))
            zero_outs.append(np.zeros(shape, dtype))
    n_params = len(in_names)
    n_outs = len(out_avals)
    all_names = in_names + out_names
    if partition_name is not None:
        all_names = all_names + [partition_name]
    donate = tuple(range(n_params, n_params + n_outs))

    def _body(*args):
        operands = list(args)
        if partition_name is not None:
            operands.append(partition_id_tensor())
        outs = _bass_exec_p.bind(
            *operands,
            out_avals=tuple(out_avals),
            in_names=tuple(all_names),
            out_names=tuple(out_names),
            lowering_input_output_aliases=(),
            sim_require_finite=True,
            sim_require_nnan=True,
            nc=nc,
        )
        return tuple(outs)

    devices = jax.devices()[:n_cores]
    mesh = Mesh(np.asarray(devices), ("core",))
    sharded = jax.jit(
        shard_map(
            _body,
            mesh=mesh,
            in_specs=(PartitionSpec("core"),) * (n_params + n_outs),
            out_specs=(PartitionSpec("core"),) * n_outs,
            check_rep=False,
        ),
        donate_argnums=donate,
        keep_unused=True,
    )

    def prep(in_maps):
        concat_in = [
            np.concatenate([np.asarray(m[name]) for m in in_maps], axis=0)
            for name in in_names
        ]
        return [jax.device_put(a) for a in concat_in]

    def run_dev(dev_in):
        concat_zeros = [
            np.zeros((n_cores * z.shape[0], *z.shape[1:]), z.dtype)
            for z in zero_outs
        ]
        out_arrs = sharded(*dev_in, *concat_zeros)
        out_arrs = [np.asarray(a) for a in out_arrs]
        return [
            {
                name: out_arrs[i].reshape(n_cores, *out_avals[i].shape)[c]
                for i, name in enumerate(out_names)
            }
            for c in range(n_cores)
        ]

    def run(in_maps):
        return run_dev(prep(in_maps))

    run.prep = prep
    run.run_dev = run_dev
    return run


_RUNNER = None


def kernel(P, d_hw, circuit_edge_pairs, circuit_edge_weights, _want_results=False):
    global _RUNNER
    in_maps = _shard_inputs(P, d_hw, circuit_edge_pairs, circuit_edge_weights)
    try:
        if _RUNNER is None:
            _RUNNER = make_runner(_get_built())
        results = _RUNNER(in_maps)
        res = None
    except Exception:
        if _want_results:
            raise
        # fallback: the stock SPMD runner (recompiles per call, but robust)
        from concourse.bass_utils import run_bass_kernel_spmd

        res = run_bass_kernel_spmd(
            _get_built(), in_maps, core_ids=list(range(NCORES))
        )
        results = res.results
    out = _combine(results)
    if _want_results:
        return out, res
    return out


# revision 25
# speedup vs baseline: 2.8789x; 2.0682x over previous
"""Trainium2 Bass kernel for AdjacencyMatchingLoss (8-core SPMD).

Math: adj_score[b,e] = P[b,i_e,:] @ A @ P[b,j_e,:]  with A = (d_hw==1).
Let W[i,j] = sum_e w_e * 1[i_e=i] * 1[j_e=j]   (weighted pair histogram)
and Gm = sum_b P_b A P_b^T scaled by -1/8 (sign + batch mean folded into
the A mask). Then the per-core partial numerator is <W, Gm>.

Structure (v2 — "Gm-first + host one-hot streaming"):
- Host ships PT (P transposed to [q, b*l], bf16). With A in natural [q,r]
  layout, Z_b = matmul(lhsT=Asc, rhs=PT_b) = (P_b A)^T and
  G_b = matmul(lhsT=Z_b, rhs=PT_b) = P_b A P_b^T — no on-device
  transposes. Gm accumulates over b in one PSUM group while the one-hot
  stream is still in flight.
- Host ships the edge one-hots directly as fp8e4m3 ([e-chunk layout,
  128-wide rows]): OhIW carries w * onehot(i), OhJ carries onehot(j)
  (exact 0/1 in fp8). One fused tensor, piece-interleaved
  [IW_p | J_p | ...] so each DMA piece delivers both matmul operands for
  a run of chunks; the PE consumes pieces as they land using DoubleRow
  fp8 matmuls (K=256: two 128-edge chunks per instruction, 0.5
  cycles/row).
- Tail: <W, Gm> = one DVE multiply (W in PSUM x Gm in SBUF) + one
  reduce, then a [128,2] partials DMA ([numerator partial, sum(w)]);
  the host sums partials over partitions/cores and divides (that
  reduction is part of unsharding the scalar output).

The w values ride inside OhIW in fp8 (~2% per-edge rounding, random
sign, averages out over 50k edges: final rel err ~1e-4). P in bf16.

This replaced a DVE-built one-hot design (21us of DVE TensorTensor at
1x — broadcast operands disqualify the 2x/4x DVE modes). CoreSim for
this version predicts ~7us vs 29.5us for the old one.
"""

import os
import sys

import numpy as np

for _p in ("/opt/trn_rl_repo",):
    if os.path.isdir(_p) and _p not in sys.path:
        sys.path.insert(0, _p)

B, NL, NQ, E = 8, 128, 128, 50000
NCORES = 8
ESH = E // NCORES            # 6250 edges per core
CHUNKS = (ESH + 127) // 128  # 49
EPAD = CHUNKS * 128          # 6272

# one-hot stream pieces, in chunk-PAIR units (DoubleRow consumes pairs);
# last piece has the odd single chunk appended
PIECE_CHUNKS = [(0, 14), (14, 26), (26, 38), (38, 49)]

PT_FP8 = True
DUAL_DMA = True  # issue oh pieces alternately on SP + ACT HWDGE rings
# pm_in packs PT + meta into ONE byte tensor [128, PM_W] (single DMA):
#   PT_FP8: [0:1024)B PT fp8 | [1024:1122)B w bf16 | [1122:1250)B d int8
#   else:   [0:2048)B PT bf16 | [2048:2146)B w bf16 | [2146:2274)B d int8
PT_B = 1024 if PT_FP8 else 2048
PM_W = PT_B + 98 + 128
# SBUF double-buffered so rep n+1's DMAs overlap rep n's compute; PSUM
# stays single-buffered (bufs=2 PSUM was HW-incorrect at high rep counts
# despite passing CoreSim — see transcript; 8/8 banks + cross-rep
# accumulation-group interleaving is the suspect).
SBUF_BUFS = 2
PSUM_BUFS = 1
DIAG = None  # None | 'dma' (DMAs only) | 'nogm' (skip Gm chain)

_BUILT = None


def _emit_body(nc, sp, pp, tensors):
    import concourse.mybir as mybir

    f32 = mybir.dt.float32
    bf16 = mybir.dt.bfloat16
    i16 = mybir.dt.int16
    i8 = mybir.dt.int8
    fp8 = mybir.dt.float8e4
    EQ = mybir.AluOpType.is_equal
    MUL = mybir.AluOpType.mult
    ADD = mybir.AluOpType.add
    DR = mybir.MatmulPerfMode.DoubleRow
    pm_d, oh_d, o_d = tensors

    pm = sp.tile([128, PM_W], i8)
    oh = sp.tile([128, 2 * EPAD], fp8)
    Asc = sp.tile([128, NQ], bf16)
    Zsb = sp.tile([128, B * NL], bf16)
    GmS = sp.tile([128, NL], bf16)
    scr = sp.tile([128, NL], f32)
    prt = sp.tile([128, 2], f32)

    Zps = pp.tile([128, B * NL], f32)
    Gps = pp.tile([128, NL], f32)
    Wps = pp.tile([128, NL], f32)

    # ---- DMAs (emitted up front so transfers stream back-to-back) ----
    # Two HWDGE rings exist on HW (SP + ACT); alternating the oh pieces
    # across them runs the two streams in parallel (CoreSim's exclusive
    # DMA device won't show the gain, HW does).
    nc.sync.dma_start(out=pm[:], in_=pm_d.ap())
    piece_off = []
    off = 0
    for pi, (c0, c1) in enumerate(PIECE_CHUNKS):
        sz = (c1 - c0) * 128
        piece_off.append(off)
        eng = nc.scalar if (DUAL_DMA and pi % 2 == 0) else nc.sync
        eng.dma_start(
            out=oh[:, off : off + 2 * sz], in_=oh_d.ap()[:, off : off + 2 * sz]
        )
        off += 2 * sz

    def iw_ap(c):
        for (c0, c1), po in zip(PIECE_CHUNKS, piece_off):
            if c0 <= c < c1:
                return po + (c - c0) * 128
        raise AssertionError(c)

    def j_ap(c):
        for (c0, c1), po in zip(PIECE_CHUNKS, piece_off):
            if c0 <= c < c1:
                return po + (c1 - c0) * 128 + (c - c0) * 128
        raise AssertionError(c)

    # views into pm (byte offsets)
    PT = pm[:, 0:PT_B].bitcast(fp8 if PT_FP8 else bf16)  # [128, 1024]
    wT = pm[:, PT_B : PT_B + 98].bitcast(bf16)           # [128, 49]
    dsb = pm[:, PT_B + 98 : PT_B + 226]                  # [128, 128] int8

    if DIAG == "dma":
        # DMA-floor probe: touch the inputs with one cheap op, then out
        nc.vector.tensor_reduce(
            out=prt[:, 1:2], in_=wT, axis=mybir.AxisListType.X, op=ADD
        )
        nc.vector.tensor_reduce(
            out=prt[:, 0:1], in_=oh[:, 0:128].bitcast(i8),
            axis=mybir.AxisListType.X, op=ADD,
        )
        nc.sync.dma_start(out=o_d.ap(), in_=prt[:])
        return

    # ---- small prep ----
    # Asc = -(1/8) * (d_hw == 1): folds sign + batch-mean into the mask
    nc.gpsimd.tensor_scalar(
        out=Asc[:], in0=dsb, scalar1=1, scalar2=-0.125, op0=EQ, op1=MUL
    )
    nc.vector.tensor_reduce(
        out=prt[:, 1:2], in_=wT, axis=mybir.AxisListType.X, op=ADD
    )

    # ---- Gm = sum_b P_b Asc P_b^T via PT-only matmuls ----
    # stage 1 as two 512-wide matmuls: same streamed rows, 4x fewer
    # stationary (Asc) reloads than 8x128
    if DIAG != "nogm":
        for h in range(2):
            sl = slice(h * 512, (h + 1) * 512)
            nc.tensor.matmul(
                Zps[:, sl], lhsT=Asc[:], rhs=PT[:, sl], start=True, stop=True
            )
        nc.vector.tensor_copy(out=Zsb[:, 0:512], in_=Zps[:, 0:512])
        nc.scalar.copy(out=Zsb[:, 512:1024], in_=Zps[:, 512:1024])
        for b in range(B):
            sl = slice(b * 128, (b + 1) * 128)
            nc.tensor.matmul(
                Gps[:], lhsT=Zsb[:, sl], rhs=PT[:, sl],
                start=(b == 0), stop=(b == B - 1),
            )
        nc.scalar.copy(out=GmS[:], in_=Gps[:])

    # ---- W accumulation from the one-hot stream ----
    if DIAG == "now":
        nc.vector.tensor_tensor(out=scr[:], in0=Gps[:], in1=GmS[:], op=MUL)
        nc.vector.tensor_reduce(
            out=prt[:, 0:1], in_=scr[:], axis=mybir.AxisListType.X, op=ADD
        )
        nc.sync.dma_start(out=o_d.ap(), in_=prt[:])
        return
    c = 0
    while c < CHUNKS:
        if c + 1 < CHUNKS and iw_ap(c + 1) == iw_ap(c) + 128:
            two = lambda a: oh[:, a : a + 256].rearrange(
                "p (two m) -> p two m", two=2
            )
            nc.tensor.matmul(
                Wps[:], lhsT=two(iw_ap(c)), rhs=two(j_ap(c)),
                start=(c == 0), stop=(c + 2 >= CHUNKS),
                perf_mode=DR,
            )
            c += 2
        else:
            nc.tensor.matmul(
                Wps[:], lhsT=oh[:, iw_ap(c) : iw_ap(c) + 128],
                rhs=oh[:, j_ap(c) : j_ap(c) + 128],
                start=(c == 0), stop=(c + 1 >= CHUNKS),
            )
            c += 1

    # ---- tail: partial = sum_j W[p,j] * Gm[p,j] ----
    gm_in = Asc[:] if DIAG == "nogm" else GmS[:]
    nc.vector.tensor_tensor(out=scr[:], in0=Wps[:], in1=gm_in, op=MUL)
    nc.vector.tensor_reduce(
        out=prt[:, 0:1], in_=scr[:], axis=mybir.AxisListType.X, op=ADD
    )
    nc.sync.dma_start(out=o_d.ap(), in_=prt[:])


def _build(reps=1):
    import concourse.bacc as bacc
    import concourse.mybir as mybir
    import concourse.tile as tile

    f32 = mybir.dt.float32
    bf16 = mybir.dt.bfloat16
    i16 = mybir.dt.int16
    fp8 = mybir.dt.float8e4

    nc = bacc.Bacc("TRN2", target_bir_lowering=False, debug=False, num_devices=NCORES)

    pm_d = nc.dram_tensor("pm_in", [128, PM_W], mybir.dt.int8, kind="ExternalInput")
    oh_d = nc.dram_tensor("oh_in", [128, 2 * EPAD], fp8, kind="ExternalInput")
    o_d = nc.dram_tensor("out", [128, 2], f32, kind="ExternalOutput")

    with tile.TileContext(nc) as tc:
        with (
            tc.tile_pool(name="sbuf", bufs=SBUF_BUFS) as sp,
            tc.tile_pool(name="psum", bufs=PSUM_BUFS, space="PSUM") as pp,
        ):
            for _ in range(reps):
                _emit_body(nc, sp, pp, (pm_d, oh_d, o_d))

    nc.compile()
    return nc


def _get_built():
    global _BUILT
    if _BUILT is None:
        _BUILT = _build()
    return _BUILT


def _shard_inputs(P, d_hw, circuit_edge_pairs, circuit_edge_weights):
    import ml_dtypes

    bf16 = ml_dtypes.bfloat16
    fp8 = ml_dtypes.float8_e4m3

    P = np.asarray(P, dtype=np.float32)
    d = np.asarray(d_hw, dtype=np.int32)
    pairs = np.asarray(circuit_edge_pairs).astype(np.int64, copy=False)
    w = np.asarray(circuit_edge_weights, dtype=np.float32)

    # PT[q, b*128 + l] = P[b, l, q]  (replicated to all cores)
    PT = np.ascontiguousarray(P.transpose(2, 0, 1).reshape(128, B * NL))

    pairs_pad = np.zeros((NCORES, EPAD, 2), dtype=np.int64)
    w_pad = np.zeros((NCORES, EPAD), dtype=np.float32)
    pairs_pad[:, :ESH] = pairs.reshape(NCORES, ESH, 2)
    w_pad[:, :ESH] = w.reshape(NCORES, ESH)

    # edge k (per core) -> chunk cc = k//128, partition p = k%128
    k = np.arange(EPAD)
    cc = k // 128
    p = k % 128
    i_idx = pairs_pad[:, :, 0].astype(np.int64)
    j_idx = pairs_pad[:, :, 1].astype(np.int64)
    core = np.repeat(np.arange(NCORES), EPAD).reshape(NCORES, EPAD)
    pp_b = np.broadcast_to(p, (NCORES, EPAD))

    ohiw_full = np.zeros((NCORES, 128, EPAD), dtype=fp8)
    ohj_full = np.zeros((NCORES, 128, EPAD), dtype=fp8)
    ohiw_full[core, pp_b, cc * 128 + i_idx] = w_pad.astype(fp8)
    ohj_full[core, pp_b, cc * 128 + j_idx] = fp8(1.0)

    # fuse into piece-interleaved layout [IW_piece | J_piece | ...]
    oh = np.zeros((NCORES, 128, 2 * EPAD), dtype=fp8)
    off = 0
    for c0, c1 in PIECE_CHUNKS:
        sz = (c1 - c0) * 128
        oh[:, :, off : off + sz] = ohiw_full[:, :, c0 * 128 : c1 * 128]
        oh[:, :, off + sz : off + 2 * sz] = ohj_full[:, :, c0 * 128 : c1 * 128]
        off += 2 * sz

    # pm: PT | w | d_hw(int8), byte layout
    w_b = w_pad.reshape(NCORES, CHUNKS, 128).transpose(0, 2, 1).astype(bf16)
    d8 = d.astype(np.int8)  # values 0..3 fit

    pm = np.zeros((NCORES, 128, PM_W), dtype=np.int8)
    pm[:, :, 0:PT_B] = PT.astype(fp8 if PT_FP8 else bf16).view(np.int8)[None]
    pm[:, :, PT_B : PT_B + 98] = np.ascontiguousarray(w_b).view(np.int8)
    pm[:, :, PT_B + 98 : PT_B + 226] = d8.view(np.int8)[None]

    return [
        {
            "pm_in": np.ascontiguousarray(pm[i]),
            "oh_in": np.ascontiguousarray(oh[i]),
        }
        for i in range(NCORES)
    ]


def _combine(results):
    parts = np.stack([np.asarray(results[i]["out"]) for i in range(NCORES)])
    numer = float(parts[:, :, 0].astype(np.float64).sum())
    wsum = float(parts[:, :, 1].astype(np.float64).sum())
    return np.float32(numer / max(wsum, 1e-8))


def make_runner(nc, n_cores=NCORES):
    """jit-once mirror of bass2jax.run_bass_via_pjrt's multi-core branch so
    repeated kernel() calls reuse the compiled NEFF."""
    import jax
    import concourse.mybir as mybir
    from concourse.bass2jax import (
        Mesh,
        PartitionSpec,
        _bass_exec_p,
        install_neuronx_cc_hook,
        partition_id_tensor,
        shard_map,
    )

    install_neuronx_cc_hook()
    partition_name = nc.partition_id_tensor.name if nc.partition_id_tensor else None

    in_names, out_names, out_avals, zero_outs = [], [], [], []
    for alloc in nc.m.functions[0].allocations:
        if not isinstance(alloc, mybir.MemoryLocationSet):
            continue
        name = alloc.memorylocations[0].name
        if alloc.kind == "ExternalInput":
            if name != partition_name:
                in_names.append(name)
        elif alloc.kind == "ExternalOutput":
            shape = tuple(alloc.tensor_shape)
            dtype = mybir.dt.np(alloc.dtype)
            out_names.append(name)
            out_avals.append(jax.core.ShapedArray(shape, dtype))
            zero_outs.append(np.zeros(shape, dtype))
    n_params = len(in_names)
    n_outs = len(out_avals)
    all_names = in_names + out_names
    if partition_name is not None:
        all_names = all_names + [partition_name]
    donate = tuple(range(n_params, n_params + n_outs))

    def _body(*args):
        operands = list(args)
        if partition_name is not None:
            operands.append(partition_id_tensor())
        outs = _bass_exec_p.bind(
            *operands,
            out_avals=tuple(out_avals),
            in_names=tuple(all_names),
            out_names=tuple(out_names),
            lowering_input_output_aliases=(),
            sim_require_finite=True,
            sim_require_nnan=True,
            nc=nc,
        )
        return tuple(outs)

    devices = jax.devices()[:n_cores]
    mesh = Mesh(np.asarray(devices), ("core",))
    sharded = jax.jit(
        shard_map(
            _body,
            mesh=mesh,
            in_specs=(PartitionSpec("core"),) * (n_params + n_outs),
            out_specs=(PartitionSpec("core"),) * n_outs,
            check_rep=False,
        ),
        donate_argnums=donate,
        keep_unused=True,
    )

    def prep(in_maps):
        concat_in = [
            np.concatenate([np.asarray(m[name]) for m in in_maps], axis=0)
            for name in in_names
        ]
        return [jax.device_put(a) for a in concat_in]

    def run_dev(dev_in):
        concat_zeros = [
            np.zeros((n_cores * z.shape[0], *z.shape[1:]), z.dtype)
            for z in zero_outs
        ]
        out_arrs = sharded(*dev_in, *concat_zeros)
        out_arrs = [np.asarray(a) for a in out_arrs]
        return [
            {
                name: out_arrs[i].reshape(n_cores, *out_avals[i].shape)[c]
                for i, name in enumerate(out_names)
            }
            for c in range(n_cores)
        ]

    def run(in_maps):
        return run_dev(prep(in_maps))

    run.prep = prep
    run.run_dev = run_dev
    return run


_RUNNER = None


def kernel(P, d_hw, circuit_edge_pairs, circuit_edge_weights, _want_results=False):
    global _RUNNER
    in_maps = _shard_inputs(P, d_hw, circuit_edge_pairs, circuit_edge_weights)
    try:
        if _RUNNER is None:
            _RUNNER = make_runner(_get_built())
        results = _RUNNER(in_maps)
        res = None
    except Exception:
        if _want_results:
            raise
        # fallback: the stock SPMD runner (recompiles per call, but robust)
        from concourse.bass_utils import run_bass_kernel_spmd

        res = run_bass_kernel_spmd(
            _get_built(), in_maps, core_ids=list(range(NCORES))
        )
        results = res.results
    out = _combine(results)
    if _want_results:
        return out, res
    return out
